# revision 4
# baseline (speedup 1.0000x reference)
"""CoordinateDecoder LSTM on 8 trn2 NeuronCores.

Strategy: 8-way tensor-parallel split of the 4H gate dimension of the LSTM.
Each core holds a (4H/8 = 1024)-row slice of W_ih/W_hh (bf16, SBUF-resident;
fp32 W_hh would not fit in SBUF and streaming it from HBM each step would be
~3x slower than compute), computes its gate slice for the FULL batch
(moving N=512 keeps the PE at ~90% of bf16 peak), updates its H/8 = 256-row
slice of c/h state in fp32, then AllGathers the new h (bf16) so every core
has the full hidden state for the next step. The h slice is gathered in two
128-row chunks so the gather of chunk0 overlaps the matmuls that only need
chunk1 (and vice versa). coord/stop projections are computed as per-core
partials (2 matmuls, M=3) and summed after riding along with the chunk-1
AllGather, so x_{t+1} = coord_t is replicated without extra collectives.

Gate m-tile order per core k: [i0,f0,g0,o0,i1,f1,g1,o1] where xP = rows
[256k+128P, 256k+128P+128) of gate x. Contraction (K) order matches the
AllGather concat order: k-tile j <-> h rows [256*(j%8) + 128*(j//8), +128).
"""
import numpy as np
import ml_dtypes

import concourse.bass as bass
import concourse.bacc as bacc
import concourse.tile as tile
import concourse.mybir as mybir
from concourse.bass_utils import run_bass_kernel_spmd

F32 = mybir.dt.float32
BF16 = mybir.dt.bfloat16
BF = ml_dtypes.bfloat16
NC = 8
H = 2048
E = 2048
B = 512
P = 128
KT = H // P            # 16 k-tiles
MT = 8                 # m-tiles per core
RG = [list(range(NC))]

_cache = {}


def _build(T):
    nc = bacc.Bacc("TRN2", target_bir_lowering=False, debug=False, num_devices=NC)

    whh_d = nc.dram_tensor("whh", [H, MT * P], BF16, kind="ExternalInput")
    wih_d = nc.dram_tensor("wih", [2, MT * P], BF16, kind="ExternalInput")
    wemb_d = nc.dram_tensor("wemb", [E, 2 * P], BF16, kind="ExternalInput")
    wos_d = nc.dram_tensor("wos", [2 * P, 3], BF16, kind="ExternalInput")
    bgate_d = nc.dram_tensor("bgate", [P, MT], F32, kind="ExternalInput")
    bemb_d = nc.dram_tensor("bemb", [P, 2], F32, kind="ExternalInput")
    bos_d = nc.dram_tensor("bos", [3, 1], F32, kind="ExternalInput")
    embT_d = nc.dram_tensor("embT", [E, B], BF16, kind="ExternalInput")
    out_d = nc.dram_tensor("out", [T, 3, B], F32, kind="ExternalOutput")

    AF = mybir.ActivationFunctionType
    GATE_FN = [AF.Sigmoid, AF.Sigmoid, AF.Tanh, AF.Sigmoid] * 2  # i,f,g,o per piece

    with tile.TileContext(nc) as tc:
        with (
            tc.tile_pool(name="w", bufs=1) as wp,
            tc.tile_pool(name="st", bufs=1) as stp_,
            tc.tile_pool(name="work", bufs=2) as wk,
            tc.tile_pool(name="psA", bufs=4, space="PSUM") as psA,
            tc.tile_pool(name="psB", bufs=3, space="PSUM") as psB,
            tc.tile_pool(name="psC", bufs=1, space="PSUM") as psC,
            tc.tile_pool(name="dram", bufs=2, space="DRAM") as dr,
        ):
            # --- resident weights/biases ---
            whh = wp.tile([P, KT, MT * P], BF16, tag="whh")
            nc.sync.dma_start(whh[:], whh_d.ap().rearrange("(kt p) m -> p kt m", p=P))
            wih = wp.tile([2, MT * P], BF16, tag="wih")
            nc.sync.dma_start(wih[:], wih_d[:])
            wemb = wp.tile([P, KT, 2 * P], BF16, tag="wemb")
            nc.sync.dma_start(wemb[:], wemb_d.ap().rearrange("(kt p) m -> p kt m", p=P))
            wos = wp.tile([P, 2, 3], BF16, tag="wos")
            nc.sync.dma_start(wos[:], wos_d.ap().rearrange("(k p) t -> p k t", p=P))
            bgate = wp.tile([P, MT], F32, tag="bgate")
            nc.sync.dma_start(bgate[:], bgate_d[:])
            bemb = wp.tile([P, 2], F32, tag="bemb")
            nc.sync.dma_start(bemb[:], bemb_d[:])
            bos = wp.tile([3, 1], F32, tag="bos")
            nc.sync.dma_start(bos[:], bos_d[:])
            embT = wp.tile([P, KT, B], BF16, tag="embT")
            nc.sync.dma_start(embT[:], embT_d.ap().rearrange("(kt p) n -> p kt n", p=P))

            # --- state ---
            c = stp_.tile([P, 2, B], F32, tag="c")
            nc.gpsimd.memset(c[:], 0.0)

            def gather(piece_tile, tag, extra=None):
                """AllGather one 128-row h piece (+optional 3-row partials)."""
                rows = P + (3 if extra is not None else 0)
                gin = dr.tile([rows, B], BF16, tag=f"{tag}i", bufs=2)
                nc.scalar.dma_start(gin[0:P, :], piece_tile[:])
                if extra is not None:
                    nc.scalar.dma_start(gin[P:P + 3, :], extra[:])
                gout = dr.tile([rows * NC, B], BF16, tag=f"{tag}o", bufs=2,
                               addr_space="Shared")
                nc.gpsimd.collective_compute(
                    "AllGather", mybir.AluOpType.bypass, replica_groups=RG,
                    ins=[gin.opt()], outs=[gout.opt()],
                )
                hg = wk.tile([P, NC, B], BF16, tag=f"{tag}g", bufs=2)
                nc.sync.dma_start(
                    hg[:], gout.rearrange("(r p) n -> p r n", p=rows)[0:P, :, :])
                pg = None
                if extra is not None:
                    pg = wk.tile([3, NC, B], BF16, tag="pg", bufs=2)
                    nc.sync.dma_start(
                        pg[:], gout.rearrange("(r p) n -> p r n", p=rows)[P:P + 3, :, :])
                return hg, pg

            def epilogue(t, pg):
                """Sum coord/stop partials of step t, write outputs, make x(t+1)."""
                s = [None] * 4
                for q in range(4):
                    s[q] = wk.tile([3, B], F32, tag="eps", bufs=8, name=f"eps{q}")
                    nc.vector.tensor_add(s[q][:], pg[:, 2 * q, :], pg[:, 2 * q + 1, :])
                s01 = wk.tile([3, B], F32, tag="eps", bufs=8)
                nc.vector.tensor_add(s01[:], s[0][:], s[1][:])
                s23 = wk.tile([3, B], F32, tag="eps", bufs=8)
                nc.vector.tensor_add(s23[:], s[2][:], s[3][:])
                cs = wk.tile([3, B], F32, tag="cs", bufs=2)
                nc.vector.tensor_add(cs[:], s01[:], s23[:])
                csb = wk.tile([3, B], F32, tag="csb", bufs=2)
                nc.scalar.activation(csb[:], cs[:], AF.Identity, bias=bos[:, 0:1])
                # row 2 is the raw stop logit; sigmoid applied on host
                nc.sync.dma_start(out_d[t, :, :], csb[:])
                xb = wk.tile([2, B], BF16, tag="xb", bufs=2)
                nc.vector.tensor_copy(xb[:], csb[0:2, :])
                return xb

            # --- h0 = W_embed_slice @ emb.T + b_embed ---
            h0p = []
            for p2 in range(2):
                ps = psA.tile([P, B], F32, tag="psA")
                for j in range(KT):
                    nc.tensor.matmul(ps[:], wemb[:, j, p2 * P:(p2 + 1) * P],
                                     embT[:, j, :], start=(j == 0), stop=(j == KT - 1))
                hp = wk.tile([P, B], BF16, tag="hp", bufs=4)
                nc.scalar.activation(hp[:], ps[:], AF.Identity, bias=bemb[:, p2:p2 + 1])
                h0p.append(hp)
            hg0, _ = gather(h0p[0], "a0")
            hg1, _ = gather(h0p[1], "a1")

            xb = None
            pg_prev = None
            t_prev = None
            for t in range(T):
                # ---- phase A: chunk-0 contraction for all 8 m-tiles ----
                gaccs = []
                for m in range(MT):
                    ps = psA.tile([P, B], F32, tag="psA")
                    for j in range(NC):
                        nc.tensor.matmul(ps[:], whh[:, j, m * P:(m + 1) * P],
                                         hg0[:, j, :], start=(j == 0), stop=(j == NC - 1))
                    ga = wk.tile([P, B], F32, tag="gacc", bufs=8)
                    nc.vector.tensor_copy(ga[:], ps[:])
                    gaccs.append(ga)

                # ---- previous step's outputs (placed here so their engine work
                # does not sit in front of this step's phase-A ops) ----
                if pg_prev is not None:
                    xb = epilogue(t_prev, pg_prev)

                # ---- phase B: x-gates + chunk-1 contraction, act, c/h ----
                acts = []
                hps = []
                for m in range(MT):
                    ps = psB.tile([P, B], F32, tag="psB")
                    first = True
                    if xb is not None:
                        nc.tensor.matmul(ps[:], wih[:, m * P:(m + 1) * P], xb[:],
                                         start=True, stop=False)
                        first = False
                    for j in range(NC):
                        nc.tensor.matmul(ps[:], whh[:, NC + j, m * P:(m + 1) * P],
                                         hg1[:, j, :], start=(first and j == 0),
                                         stop=(j == NC - 1))
                    gs = wk.tile([P, B], F32, tag="gs", bufs=4)
                    nc.vector.tensor_add(gs[:], ps[:], gaccs[m][:])
                    at = wk.tile([P, B], F32, tag="act", bufs=8)
                    nc.scalar.activation(at[:], gs[:], GATE_FN[m], bias=bgate[:, m:m + 1])
                    acts.append(at)

                    if m in (3, 7):
                        p2 = m // 4
                        ai, af_, ag_, ao = acts[4 * p2:4 * p2 + 4]
                        t1 = wk.tile([P, B], F32, tag="tmp", bufs=6)
                        nc.vector.tensor_mul(t1[:], ai[:], ag_[:])
                        t2 = wk.tile([P, B], F32, tag="tmp", bufs=6)
                        nc.vector.tensor_mul(t2[:], af_[:], c[:, p2, :])
                        nc.vector.tensor_add(c[:, p2, :], t1[:], t2[:])
                        t3 = wk.tile([P, B], F32, tag="tmp", bufs=6)
                        nc.scalar.activation(t3[:], c[:, p2, :], AF.Tanh)
                        hp = wk.tile([P, B], BF16, tag="hp", bufs=4)
                        nc.vector.tensor_mul(hp[:], ao[:], t3[:])
                        hps.append(hp)
                        if p2 == 0:
                            hg0, _ = gather(hp, "a0")
                        else:
                            cp = psC.tile([3, B], F32, tag="psC")
                            nc.tensor.matmul(cp[:], wos[:, 0, :], hps[0][:],
                                             start=True, stop=False)
                            nc.tensor.matmul(cp[:], wos[:, 1, :], hps[1][:],
                                             start=False, stop=True)
                            pb = wk.tile([3, B], BF16, tag="pb", bufs=2)
                            nc.vector.tensor_copy(pb[:], cp[:])
                            hg1, pg_prev = gather(hp, "a1", extra=pb)
                t_prev = t

            epilogue(t_prev, pg_prev)

    nc.compile()
    return nc


def _prep_inputs(inputs):
    W_embed = np.asarray(inputs["W_embed"], np.float32)
    b_embed = np.asarray(inputs["b_embed"], np.float32)
    W_ih = np.asarray(inputs["W_ih"], np.float32)
    b_ih = np.asarray(inputs["b_ih"], np.float32)
    W_hh = np.asarray(inputs["W_hh"], np.float32)
    b_hh = np.asarray(inputs["b_hh"], np.float32)
    W_out = np.asarray(inputs["W_out"], np.float32)
    b_out = np.asarray(inputs["b_out"], np.float32)
    W_stop = np.asarray(inputs["W_stop"], np.float32)
    b_stop = np.asarray(inputs["b_stop"], np.float32)
    emb = np.asarray(inputs["embedding"], np.float32)

    kidx = np.concatenate(
        [np.arange(256 * (j % 8) + 128 * (j // 8),
                   256 * (j % 8) + 128 * (j // 8) + 128) for j in range(KT)])
    embT = np.ascontiguousarray(emb.T).astype(BF)
    wos_full = np.concatenate([W_out, W_stop], axis=0)          # (3, H)
    bos = np.concatenate([b_out, b_stop]).reshape(3, 1).astype(np.float32)
    bg_full = b_ih + b_hh

    in_maps = []
    for k in range(NC):
        gsel = np.concatenate(
            [np.arange(2048 * g + 256 * k + 128 * p2,
                       2048 * g + 256 * k + 128 * p2 + 128)
             for p2 in range(2) for g in range(4)])
        in_maps.append({
            "whh": np.ascontiguousarray(W_hh[gsel][:, kidx].T).astype(BF),
            "wih": np.ascontiguousarray(W_ih[gsel].T).astype(BF),
            "wemb": np.ascontiguousarray(W_embed[256 * k:256 * k + 256].T).astype(BF),
            "wos": np.ascontiguousarray(wos_full[:, 256 * k:256 * k + 256].T).astype(BF),
            "bgate": np.ascontiguousarray(bg_full[gsel].reshape(MT, P).T).astype(np.float32),
            "bemb": np.ascontiguousarray(
                b_embed[256 * k:256 * k + 256].reshape(2, P).T).astype(np.float32),
            "bos": bos,
            "embT": embT,
        })
    return in_maps


def kernel(**inputs):
    T = int(inputs["max_seq_length"])
    if T not in _cache:
        _cache[T] = _build(T)
    nc = _cache[T]
    in_maps = _prep_inputs(inputs)
    res = run_bass_kernel_spmd(nc, in_maps, core_ids=list(range(NC)))
    out = res.results[0]["out"]                       # (T, 3, B) f32
    coords = np.ascontiguousarray(np.transpose(out[:, 0:2, :], (2, 0, 1)))
    logit = np.transpose(out[:, 2:3, :], (2, 0, 1))
    stops = np.ascontiguousarray(1.0 / (1.0 + np.exp(-logit))).astype(np.float32)
    return coords, stops
